# revision 14
# baseline (speedup 1.0000x reference)
"""Trainium2 Bass kernel for LSTM encode/decode seq2seq (nn_RNN_25409026523753).

Model (per reference):
  encode: 257 steps consuming x_proj[t] = x[:,t,:] @ W_ih.T + b_ih
  decode: 256 steps with zero input; out_t = sigmoid(h @ fc_W.T + fc_b)

Strategy:
  - Data-parallel over batch N=512 -> 8 cores x 64.
  - Per step: gates[64, 2048] = x_proj + h @ W_hh.T  computed on PE with
    batch on PSUM partitions.  Gate columns are host-permuted into two
    1024-wide blocks [i f o g] (each quarter 256 wide), the two blocks
    stacked on PSUM partitions 0-63 / 64-127 so elementwise runs at the
    full 128-lane width.  The two partition halves use PE column-group
    tiling (tile_position (0,0)/(0,64)) so their matmuls overlap in the
    array.
  - lhsT (stationary) = h.T chunks [128, 64]; cheap to reload (64 cols).
  - x projection handled inline as an extra K-pass with stationary
    x_t.T augmented with a ones-row so b_ih+b_hh ride along for free.
  - h.T for the next step is produced with 4 PE transposes + one DVE copy.
"""

import os
import sys

import numpy as np

N, T, H, D = 512, 256, 512, 9
NCORES = 8
NB = N // NCORES          # 64 batch rows per core
NENC = T + 1              # 257
NDEC = T                  # 256
G4 = 4 * H                # 2048 gate columns
HB = H // 2               # 256 c-dims per partition half
OUTW = NDEC * D           # 2304 output cols per core

_REPO = "/opt/trn_rl_repo"


def _ensure_path():
    if _REPO not in sys.path:
        sys.path.insert(0, _REPO)


def _gate_perm():
    """Column permutation: W rows [i f g o] (512 each) ->
    [i0 f0 o0 g0 | i1 f1 o1 g1] with 256-wide quarters."""
    i = np.arange(0, 512)
    f = np.arange(512, 1024)
    g = np.arange(1024, 1536)
    o = np.arange(1536, 2048)
    parts = []
    for half in (slice(0, 256), slice(256, 512)):
        for gate in (i, f, o, g):
            parts.append(gate[half])
    return np.concatenate(parts)


def build_program(n_enc, n_dec, unroll=16, dec_bias=False, fc_bias=False):
    """Emit the Bass program.  Returns (nc, names) where names maps logical
    tensor roles to DRAM tensor names."""
    _ensure_path()
    import concourse.bacc as bacc
    import concourse.bass as bass
    import concourse.mybir as mybir
    import concourse.tile as tile
    from contextlib import ExitStack

    f32 = mybir.dt.float32
    AF = mybir.ActivationFunctionType
    ds = bass.ds

    nc = bacc.Bacc(None, target_bir_lowering=False)

    # ---- DRAM I/O ----
    d_w4 = nc.dram_tensor("w4", [128, 4 * G4], f32, kind="ExternalInput")
    d_wih = nc.dram_tensor("wih", [D + 1, G4], f32, kind="ExternalInput")
    d_fcw = nc.dram_tensor("fcw4", [128, 4 * D], f32, kind="ExternalInput")
    d_fcb = nc.dram_tensor("fcb", [1, D], f32, kind="ExternalInput")
    d_id = nc.dram_tensor("ident", [128, 64], f32, kind="ExternalInput")
    d_ci = nc.dram_tensor("cinit", [128, HB], f32, kind="ExternalInput")
    d_hti = nc.dram_tensor("htinit", [128, HB], f32, kind="ExternalInput")
    d_xt = nc.dram_tensor("xt", [D + 1, n_enc * NB], f32, kind="ExternalInput")
    d_zst = nc.dram_tensor("zstage", [D + 1, 64], f32, kind="ExternalInput")
    d_ones = nc.dram_tensor("ones", [1, 64], f32, kind="ExternalInput")
    d_out = nc.dram_tensor("out", [NB, n_dec * D], f32, kind="ExternalOutput")

    # PSUM region map: (row0, col0, permuted-gate-col0)
    REGIONS = ((0, 0, 0), (64, 0, 1024), (0, 512, 512), (64, 512, 1536))

    with ExitStack() as ctx:
        tc = ctx.enter_context(tile.TileContext(nc))
        consts = ctx.enter_context(tc.tile_pool(name="consts", bufs=1))
        state = ctx.enter_context(tc.tile_pool(name="state", bufs=1))
        work = ctx.enter_context(tc.tile_pool(name="work", bufs=2))
        xsp = ctx.enter_context(tc.tile_pool(name="xsp", bufs=2))
        psg = ctx.enter_context(tc.tile_pool(name="psg", bufs=2, space="PSUM"))
        pst = ctx.enter_context(tc.tile_pool(name="pst", bufs=2, space="PSUM"))
        psf = ctx.enter_context(tc.tile_pool(name="psf", bufs=2, space="PSUM"))

        w4 = consts.tile([128, 4 * G4], f32)
        wih = consts.tile([D + 1, G4], f32)
        fcw = consts.tile([128, 4 * D], f32)
        fcb = consts.tile([1, D], f32)
        ident = consts.tile([128, 64], f32)
        xt = consts.tile([D + 1, n_enc * NB], f32)

        c = state.tile([128, HB], f32)
        h = state.tile([128, HB], f32)
        hT = state.tile([128, HB], f32)
        out_sb = state.tile([NB, n_dec * D], f32)
        zst = state.tile([D + 1, 64], f32, name="zst")
        ones_t = state.tile([1, 64], f32, name="ones_t")

        nc.sync.dma_start(w4[:], d_w4[:])
        nc.sync.dma_start(wih[:], d_wih[:])
        nc.sync.dma_start(fcw[:], d_fcw[:])
        nc.sync.dma_start(fcb[:], d_fcb[:])
        nc.sync.dma_start(ident[:], d_id[:])
        nc.sync.dma_start(c[:], d_ci[:])
        nc.sync.dma_start(hT[:], d_hti[:])
        nc.sync.dma_start(xt[:], d_xt[:])
        nc.sync.dma_start(zst[:], d_zst[:])
        nc.sync.dma_start(ones_t[:], d_ones[:])

        def emit_step(xstat, out_ap):
            """One LSTM step.  xstat: stationary [D+1, 64] AP for the input
            projection K-pass (None -> no x-pass).  out_ap: decode output
            [NB, D] slice of out_sb (None for encode)."""
            G = psg.tile([128, 1024], f32, tag="G")
            has_x = xstat is not None
            if has_x:
                for r0, c0, w0 in REGIONS:
                    nc.tensor.matmul(
                        G[r0 : r0 + 64, c0 : c0 + 512],
                        xstat,
                        wih[:, w0 : w0 + 512],
                        start=True,
                        stop=False,
                        tile_position=(0, r0),
                        skip_group_check=True,
                    )
            for k in range(4):
                lhs = hT[:, 64 * k : 64 * k + 64]
                for r0, c0, w0 in REGIONS:
                    nc.tensor.matmul(
                        G[r0 : r0 + 64, c0 : c0 + 512],
                        lhs,
                        w4[:, k * G4 + w0 : k * G4 + w0 + 512],
                        start=(k == 0 and not has_x),
                        stop=(k == 3),
                        tile_position=(0, r0),
                        skip_group_check=True,
                    )
            # elementwise: cols 0:256 = i, 256:512 = f, 512:768 = o, 768:1024 = g
            sig = work.tile([128, 768], f32, tag="sig")
            tg = work.tile([128, HB], f32, tag="tg")
            tch = work.tile([128, HB], f32, tag="tch")
            t1 = work.tile([128, HB], f32, tag="t1")
            nc.scalar.activation(sig[:, 0:512], G[:, 0:512], AF.Sigmoid)
            nc.scalar.activation(tg[:], G[:, 768:1024], AF.Tanh)
            nc.scalar.activation(sig[:, 512:768], G[:, 512:768], AF.Sigmoid)
            nc.vector.tensor_mul(t1[:], sig[:, 0:256], tg[:])
            nc.vector.tensor_mul(c[:], sig[:, 256:512], c[:])
            nc.vector.tensor_add(c[:], c[:], t1[:])
            nc.scalar.activation(tch[:], c[:], AF.Tanh)
            nc.vector.tensor_mul(h[:], sig[:, 512:768], tch[:])
            # h -> hT via 4 PE transposes
            PT = pst.tile([128, HB], f32, tag="PT")
            for k, (r0, c0) in enumerate(((0, 0), (0, 128), (64, 0), (64, 128))):
                nc.tensor.transpose(
                    PT[:, 64 * k : 64 * k + 64],
                    h[r0 : r0 + 64, c0 : c0 + 128],
                    ident[r0 : r0 + 64, :],
                )
                nc.vector.tensor_copy(
                    hT[:, 64 * k : 64 * k + 64], PT[:, 64 * k : 64 * k + 64]
                )
            if out_ap is not None:
                PF = psf.tile([64, D], f32, tag="PF")
                for k in range(4):
                    nc.tensor.matmul(
                        PF[:],
                        hT[:, 64 * k : 64 * k + 64],
                        fcw[:, D * k : D * k + D],
                        start=(k == 0),
                        stop=(k == 3 and not fc_bias),
                        tile_position=(0, 0),
                    )
                if fc_bias:
                    nc.tensor.matmul(
                        PF[:],
                        ones_t[:],
                        fcb[:],
                        start=False,
                        stop=True,
                        tile_position=(0, 0),
                    )
                nc.scalar.activation(out_ap, PF[:], AF.Sigmoid)

        PE = mybir.EngineType.PE

        # ---------- encode ----------
        enc_chunks = n_enc // unroll
        if enc_chunks > 0:
            with tc.For_i(0, enc_chunks, hint_engines=(PE,), name="enc") as i:
                for u in range(unroll):
                    xs = xsp.tile([D + 1, 64], f32, tag="xs")
                    nc.gpsimd.tensor_copy(
                        xs[:], xt[:, ds(i * (unroll * NB) + u * NB, NB)]
                    )
                    emit_step(xs[:], None)
        for t in range(enc_chunks * unroll, n_enc):
            emit_step(xt[:, t * NB : (t + 1) * NB], None)

        # ---------- decode ----------
        def dec_xstat():
            return zst[:] if dec_bias else None

        dec_chunks = n_dec // unroll
        if dec_chunks > 0:
            with tc.For_i(0, dec_chunks, hint_engines=(PE,), name="dec") as i:
                for u in range(unroll):
                    emit_step(dec_xstat(), out_sb[:, ds(i * (unroll * D) + u * D, D)])
        for t in range(dec_chunks * unroll, n_dec):
            emit_step(dec_xstat(), out_sb[:, t * D : (t + 1) * D])

        nc.sync.dma_start(d_out[:], out_sb[:])

    nc.compile()
    return nc


def prep_inputs(x, W_ih, W_hh, b_ih, b_hh, fc_W, fc_b, h0, c0, n_enc=NENC):
    """Host-side prep: permute/transpose weights, build per-core input maps."""
    f32 = np.float32
    x = np.asarray(x, f32)
    perm = _gate_perm()
    Wih_p = np.asarray(W_ih, f32)[perm, :]          # [2048, 9]
    Whh_p = np.asarray(W_hh, f32)[perm, :]          # [2048, 512]
    bias_p = (np.asarray(b_ih, f32) + np.asarray(b_hh, f32))[perm]  # [2048]

    wih_aug = np.concatenate([Wih_p.T, bias_p[None, :]], axis=0)  # [10, 2048]
    # w4[r, k*2048 + j] = W_hh.T[128k + r, perm j]
    w4 = np.ascontiguousarray(
        Whh_p.T.reshape(4, 128, G4).transpose(1, 0, 2).reshape(128, 4 * G4)
    )
    fcw4 = np.ascontiguousarray(
        np.asarray(fc_W, f32).T.reshape(4, 128, D).transpose(1, 0, 2).reshape(128, 4 * D)
    )
    fcb_row = np.asarray(fc_b, f32).reshape(1, D)
    ident = np.tile(np.eye(64, dtype=f32), (2, 1))  # [128, 64]

    c0 = np.asarray(c0, f32).reshape(H)
    h0 = np.asarray(h0, f32).reshape(H)
    cinit = np.empty((128, HB), f32)
    cinit[0:64, :] = c0[0:HB]
    cinit[64:128, :] = c0[HB:]
    htinit = np.empty((128, HB), f32)
    for k in range(4):
        htinit[:, 64 * k : 64 * k + 64] = h0[128 * k : 128 * (k + 1)][:, None]

    zstage = np.zeros((D + 1, 64), f32)
    zstage[D, :] = 1.0
    shared = {
        "zstage": zstage,
        "ones": np.ones((1, 64), f32),
        "w4": w4,
        "wih": np.ascontiguousarray(wih_aug),
        "fcw4": fcw4,
        "fcb": fcb_row,
        "ident": ident,
        "cinit": cinit,
        "htinit": htinit,
    }
    in_maps = []
    for core in range(NCORES):
        b0 = core * NB
        xs = x[b0 : b0 + NB, :n_enc, :]             # [64, n_enc, 9]
        xt = np.empty((D + 1, n_enc * NB), f32)
        # xt[d, t*64 + b] = x[b, t, d]
        xt[0:D, :] = xs.transpose(2, 1, 0).reshape(D, n_enc * NB)
        xt[D, :] = 1.0
        m = dict(shared)
        m["xt"] = xt
        in_maps.append(m)
    dec_bias = bool(np.any(bias_p))
    fc_bias = bool(np.any(fcb_row))
    return in_maps, dec_bias, fc_bias


def run(inputs, n_enc=NENC, n_dec=NDEC, unroll=16, trace=False):
    """Build, run on 8 cores, gather.  Returns (out [N, n_dec, D], results)."""
    _ensure_path()
    from concourse.bass_utils import run_bass_kernel_spmd

    in_maps, dec_bias, fc_bias = prep_inputs(**inputs, n_enc=n_enc)
    nc = build_program(n_enc, n_dec, unroll=unroll, dec_bias=dec_bias, fc_bias=fc_bias)
    res = run_bass_kernel_spmd(
        nc, in_maps, core_ids=list(range(NCORES)), trace=trace
    )
    out = np.empty((N, n_dec, D), np.float32)
    for core in range(NCORES):
        out[core * NB : (core + 1) * NB] = res.results[core]["out"].reshape(
            NB, n_dec, D
        )
    return out, res


def kernel(x, W_ih, W_hh, b_ih, b_hh, fc_W, fc_b, h0, c0):
    out, _ = run(
        dict(
            x=x, W_ih=W_ih, W_hh=W_hh, b_ih=b_ih, b_hh=b_hh,
            fc_W=fc_W, fc_b=fc_b, h0=h0, c0=c0,
        )
    )
    return out
